# revision 41
# baseline (speedup 1.0000x reference)
"""Causal multi-head self-attention on 8 Trainium2 NeuronCores.

Problem: B=8, T=1024, D=1024, 16 heads (H=64), fp32 in/out, causal softmax,
y = softmax(mask(q k^T)/sqrt(H)) v, then output projection. Weights are
nn.Linear style: q = x @ Wq^T etc.

Sharding: pure data-parallel - one batch element per core, weights
replicated, no collectives.

v2 structure (single fused pipeline, PE kept warm + dense):
  - ~6us of dummy warmup matmuls at t=0 so the PE HAM clock-gate opens
    (2.4GHz) before real work arrives; they overlap the input DMAs.
  - all phases interleaved at (m-tile, 512-half) granularity: attention
    head-group/tq-block units are emitted between projection halves, and
    the output projection for tq block jq streams as soon as all four
    head groups finish that jq block, so y DMAs spread across the kernel
    instead of bunching in a tail.
  - PSUM: 2x[128,1024] score superblocks + 2x[65,512] attV accumulators
    + 2x[128,512] projection slots = 8 banks.
  - attention inner loop consumes each exp'd superblock immediately
    (4 attV matmuls right after the exp), with projection halves pumped
    between superblocks to cover the ACT-engine latency.

Per-core layout (all feature-major, zero on-device transposes); matmul
operands stored in bf16 (fp32 PSUM accumulation), fp32 output:
  host sends xT = x[b].T  [d, t]  and W*T = W*.T  [d_in, d_out]
  qT[do,t] = sum_d WqT[d,do] * xT[d,t]   (lhsT=WqT, rhs=xT)
  kT       likewise
  v[t,do]  = sum_d xT[d,t]  * WvT[d,do]  (lhsT=xT,  rhs=WvT)
  per 4-head group g, per tq block jq of 256, per tk tile i of 128:
    S^T[tk,tq] = sum_hd kT_h[hd,tk] qT_h[hd,tq]  (4 heads in a [128,1024]
                 PSUM superblock; quarters permuted so each 2KB PSUM bank
                 only receives matmuls of ONE PE quadrant position)
    E = exp(S^T/8); diagonal tiles masked (DVE mul with 0/1 mask or
    GpSimd affine_select, alternating)
    attV accumulates vp_h^T E_h over i into [65,512] PSUM pairs
    (vp = [v_h | 1] so row 64 is the softmax denominator)
  outT_h = outX[0:64] * recip(outX[64])  (flash-style denominator,
           broadcast across partitions via a DRAM-bounce DMA)
  y[t,do] = sum_d outT[d,t] * WoT[d,do]
"""

import numpy as np
from collections import deque
from contextlib import ExitStack

N_CORES = 8
T = 1024
D = 1024
NH = 16
HD = 64  # head dim
P = 128
NT = D // P  # 8 tiles of 128 along d or t
TQ = 512     # projection half width
TB = 256     # tq block width in the 4-head-packed attention
NJ = 4       # tq blocks of 256
VP = HD + 1  # v-plus: head cols + ones col
SCALE = 1.0 / 8.0  # 1/sqrt(64)

N_WARMUP = 84   # dummy matmuls at t=0: open the HAM clock gate AND keep
                # the PE continuously busy through the ~19us input-DMA
                # window (a DMA-gated matmul trickle breaks the HAM
                # activity window and leaves the PE at 1.2GHz).  Must be
                # full-width N=512 matmuls: short ones have ~50% PE duty
                # (LDW gaps) and never flip the activity monitor.

# matmul compute dtype: "fp32" (exact, 1/4 rate), "fp32r" (TF32-like, full
# rate), or "bf16" (full rate, half DMA/SBUF traffic)
MM_MODE = "bf16"

_COMPILED = {}


def _build(nc, tile, mybir, mm_dt):
    """Emit the per-core Tile program into nc."""
    f32 = mybir.dt.float32
    Exp = mybir.ActivationFunctionType.Exp

    mdt = mm_dt  # storage dtype for matmul operands
    xT_d = nc.declare_dram_parameter("xT", [D, T], mdt, isOutput=False)
    wqT_d = nc.declare_dram_parameter("wqT", [D, D], mdt, isOutput=False)
    wkT_d = nc.declare_dram_parameter("wkT", [D, D], mdt, isOutput=False)
    wvT_d = nc.declare_dram_parameter("wvT", [D, D], mdt, isOutput=False)
    woT_d = nc.declare_dram_parameter("woT", [D, D], mdt, isOutput=False)
    y_d = nc.declare_dram_parameter("y", [T, D], f32, isOutput=True)

    def mm(out, lhsT, rhs, start, stop):
        nc.tensor.matmul(out, lhsT, rhs, start=start, stop=stop)

    with ExitStack() as ctx:
        tc = ctx.enter_context(tile.TileContext(nc))

        # ---- resident SBUF pools ----
        pqk = ctx.enter_context(tc.tile_pool(name="pqk", bufs=16))
        pv = ctx.enter_context(tc.tile_pool(name="pv", bufs=8))
        pwq = ctx.enter_context(tc.tile_pool(name="pwq", bufs=8))
        pwkv = ctx.enter_context(tc.tile_pool(name="pwkv", bufs=3))
        pxt = ctx.enter_context(tc.tile_pool(name="pxt", bufs=8))
        pout = ctx.enter_context(tc.tile_pool(name="pout", bufs=8))
        pkm = ctx.enter_context(tc.tile_pool(name="pkm", bufs=1))
        pe = ctx.enter_context(tc.tile_pool(name="pe", bufs=4))
        pux = ctx.enter_context(tc.tile_pool(name="pux", bufs=3))
        pct = ctx.enter_context(tc.tile_pool(name="pct", bufs=2))
        pbc = ctx.enter_context(tc.tile_pool(name="pbc", bufs=3))
        py = ctx.enter_context(tc.tile_pool(name="py", bufs=2))

        qT = [pqk.tile([P, T], mdt, tag="qk", name=f"qT{i}") for i in range(NT)]
        kT = [pqk.tile([P, T], mdt, tag="qk", name=f"kT{i}") for i in range(NT)]
        vp = [pv.tile([P, NH * VP], mdt, tag="vp", name=f"vp{i}")
              for i in range(NT)]
        wq_sb = [pwq.tile([P, D], mdt, tag="wq", name=f"wq{i}")
                 for i in range(NT)]
        wk_all = pwkv.tile([P, NT * D], mdt, tag="wkv", name="wk_all")
        wv_all = pwkv.tile([P, NT * D], mdt, tag="wkv", name="wv_all")
        wo_all = pwkv.tile([P, NT * D], mdt, tag="wkv", name="wo_all")
        xT = [pxt.tile([P, T], mdt, tag="xt", name=f"xTs{i}")
              for i in range(NT)]
        outT = [pout.tile([P, T], mdt, tag="ot", name=f"outT{i}")
                for i in range(NT)]

        # ---- warmup: open the HAM clock gate while input DMAs run ----
        # (emitted first so the PE is at 2.4GHz when real matmuls start;
        # reads a memset tile, writes a scratch PSUM bank that the stack
        # allocator reuses afterwards)
        with (
            tc.tile_pool(name="pwarm", bufs=1) as pwarm,
            tc.tile_pool(name="pwarm_ps", bufs=2, space="PSUM") as pwarm_ps,
        ):
            wt = pwarm.tile([P, P + TQ], mdt, name="warm_in")
            nc.gpsimd.memset(wt[:], 0.0)
            wp = [pwarm_ps.tile([P, TQ], f32, tag="wps", name=f"wps{i}")
                  for i in range(2)]
            for i in range(N_WARMUP):
                mm(wp[i % 2][:], wt[:, 0:P], wt[:, P:P + TQ],
                   start=True, stop=True)

        # ---- input DMAs (dependency-first order) ----
        nc.sync.dma_start(out=wq_sb[0][:, 0:P], in_=wqT_d[0:P, 0:P])
        nc.sync.dma_start(out=xT[0][:, 0:TQ], in_=xT_d[0:P, 0:TQ])
        nc.sync.dma_start(out=wq_sb[0][:, P:D], in_=wqT_d[0:P, P:D])
        nc.sync.dma_start(out=xT[0][:, TQ:T], in_=xT_d[0:P, TQ:T])
        for k in range(1, NT):
            nc.sync.dma_start(out=wq_sb[k][:], in_=wqT_d[k * P:(k + 1) * P, :])
            nc.sync.dma_start(out=xT[k][:], in_=xT_d[k * P:(k + 1) * P, :])
        nc.sync.dma_start(
            out=wk_all.rearrange("p (k d) -> p k d", d=D),
            in_=wkT_d.rearrange("(k p) d -> p k d", p=P),
        )
        nc.sync.dma_start(
            out=wv_all.rearrange("p (k d) -> p k d", d=D),
            in_=wvT_d.rearrange("(k p) d -> p k d", p=P),
        )
        nc.sync.dma_start(
            out=wo_all.rearrange("p (k d) -> p k d", d=D),
            in_=woT_d.rearrange("(k p) d -> p k d", p=P),
        )

        # ---- 0/1 keep-mask for gm=0 diagonal tiles ----
        keep0 = pkm.tile([P, 4 * TB], mdt, name="keep0")
        nc.gpsimd.memset(keep0[:], 1.0)
        kv = keep0.rearrange("p (q f) -> p q f", q=4)
        nc.gpsimd.affine_select(
            kv, kv, pattern=[[0, 4], [1, TB]],
            compare_op=mybir.AluOpType.is_ge, fill=0.0,
            base=0, channel_multiplier=-1,
        )
        # ones columns of v-plus (denominator rows for the attV matmul)
        for m in range(NT):
            ones_cols = vp[m].rearrange("p (h c) -> p h c", c=VP)[:, :, VP - 1]
            nc.gpsimd.memset(ones_cols, 1.0)

        # ---- PSUM pools (8 banks total) ----
        pp_s = ctx.enter_context(
            tc.tile_pool(name="pp_s", bufs=2, space="PSUM"))
        pp_av = ctx.enter_context(
            tc.tile_pool(name="pp_av", bufs=2, space="PSUM"))
        pp_p = ctx.enter_context(
            tc.tile_pool(name="pp_p", bufs=2, space="PSUM"))

        # ---- copy-engine rotation for PSUM->SBUF drains ----
        cp_state = {"i": 0}

        def drain_copy(dst, src):
            # gpsimd cannot read PSUM: rotate vector/scalar 3:1
            r = cp_state["i"] % 4
            cp_state["i"] += 1
            if r == 2:
                nc.scalar.copy(dst, src)
            else:
                nc.vector.tensor_copy(dst, src)

        # ---- work-unit emitters ----
        def emit_qk_half(which, m, n):
            ps = pp_p.tile([P, TQ], f32, tag="pp")
            for k in range(NT):
                if which == "q":
                    lhsT = wq_sb[k][:, m * P:(m + 1) * P]
                else:
                    lhsT = wk_all[:, k * D + m * P:k * D + (m + 1) * P]
                mm(ps[:], lhsT, xT[k][:, n * TQ:(n + 1) * TQ],
                   start=(k == 0), stop=(k == NT - 1))
            dst = (qT if which == "q" else kT)[m][:, n * TQ:(n + 1) * TQ]
            drain_copy(dst, ps[:])

        def emit_v_half(m, n):
            # v natural layout: out rows = t tile m, cols = d half n
            ps = pp_p.tile([P, TQ], f32, tag="pp")
            for k in range(NT):
                mm(ps[:], xT[k][:, m * P:(m + 1) * P],
                   wv_all[:, k * D + n * TQ:k * D + (n + 1) * TQ],
                   start=(k == 0), stop=(k == NT - 1))
            hpn = TQ // HD  # heads per 512-half
            vdst = vp[m].rearrange("p (h c) -> p h c", c=VP)[
                :, n * hpn:(n + 1) * hpn, 0:HD]
            vsrc = ps.rearrange("p (h c) -> p h c", c=HD)
            nc.vector.tensor_copy(vdst, vsrc)

        o_open = {}

        def emit_o_mms(ps, m, n, kfrom, kend):
            for k in range(kfrom, kend):
                mm(ps[:], outT[k][:, m * P:(m + 1) * P],
                   wo_all[:, k * D + n * TQ:k * D + (n + 1) * TQ],
                   start=(k == 0), stop=(k == NT - 1))

        def emit_o_partial(m, n, kend):
            # open the y (m,n) psum and accumulate k < kend; the chain
            # parks in its pp_p buf so the k terms gated on the last head
            # group can finish after its flush without idling the PE
            ps = pp_p.tile([P, TQ], f32, tag="pp")
            emit_o_mms(ps, m, n, 0, kend)
            o_open[(m, n)] = (ps, kend)

        def emit_o_drain(ps, m, n):
            ysb = py.tile([P, TQ], f32, tag="y")
            nc.vector.tensor_copy(ysb[:], ps[:])
            nc.sync.dma_start(
                out=y_d[m * P:(m + 1) * P, n * TQ:(n + 1) * TQ], in_=ysb[:])

        def emit_o_finish(m, n):
            ps, kend = o_open.pop((m, n))
            emit_o_mms(ps, m, n, kend, NT)
            emit_o_drain(ps, m, n)

        def emit_o_half(m, n):
            ps = pp_p.tile([P, TQ], f32, tag="pp")
            emit_o_mms(ps, m, n, 0, NT)
            emit_o_drain(ps, m, n)

        # ---- filler queue: half-units pumped between attention blocks ----
        # queue holds (key, thunk); force() emits specific keys a unit
        # depends on, pump() emits from the front to fill PE gaps while
        # the ACT engine works through the exp superblocks.
        fillers = deque()
        emitted = set()

        def enq(key, thunk):
            fillers.append((key, thunk))

        fill_ctr = {"n": 0}

        def emit_dummy():
            # 2 full-width matmuls into a scores-pool bank: pure PE-array
            # activity to hold the HAM clock gate open through ACT-bound
            # stretches; the result is never read
            dps = pp_s.tile([P, 4 * TB], f32, tag="ps", name="dummy_ps")
            for _ in range(2):
                mm(dps[:, 0:TQ], keep0[:, 0:P], keep0[:, P:P + TQ],
                   start=True, stop=True)

        def maybe_fill(stride, dummy_ok):
            # once per superblock: every stride-th slot emits a real
            # filler half; other slots emit dummies when allowed
            fill_ctr["n"] += 1
            if fillers and fill_ctr["n"] % stride == 0:
                key, thunk = fillers.popleft()
                emitted.add(key)
                thunk()
            elif dummy_ok:
                emit_dummy()

        def pump(k=1):
            for _ in range(k):
                if not fillers:
                    return
                key, thunk = fillers.popleft()
                emitted.add(key)
                thunk()

        def force(keys):
            keys = [k for k in keys if k not in emitted]
            if not keys:
                return
            want = set(keys)
            rest = deque()
            while fillers and want:
                key, thunk = fillers.popleft()
                if key in want:
                    emitted.add(key)
                    thunk()
                    want.discard(key)
                else:
                    rest.append((key, thunk))
            fillers.extendleft(reversed(rest))
            assert not want, f"missing filler deps: {want}"

        def drain_all():
            pump(len(fillers))



        def emit_attn(g, jq, pump_every=1, dummy_ok=False):
            ni = 2 * jq + 2
            avA = pp_av.tile([HD + 1, 2 * TB], f32, tag="av")
            avB = pp_av.tile([HD + 1, 2 * TB], f32, tag="av")
            av = (avA, avB)
            order = list(range(ni - 1, -1, -1))  # diagonal tiles first
            for idx, i in enumerate(order):
                ps = pp_s.tile([P, 4 * TB], f32, tag="ps")
                gm = i - 2 * jq
                for q4 in range(4):
                    it = 2 * g + q4 // 2
                    po = (q4 % 2) * HD
                    # same-quadrant matmuls share a PSUM bank: bank0 gets
                    # the two po=0 heads, bank1 the po=64 heads (mixed
                    # quadrant positions in one bank fault the hardware)
                    cs = ((q4 % 2) * 2 + q4 // 2) * TB
                    mm(ps[:, cs:cs + TB],
                       kT[it][po:po + HD, i * P:(i + 1) * P],
                       qT[it][po:po + HD, jq * TB:(jq + 1) * TB],
                       start=True, stop=True)
                e = pe.tile([P, 4 * TB], mdt, tag="e")
                nc.scalar.activation(e[:], ps[:], Exp, scale=SCALE)
                if gm == 0:  # DVE multiply path
                    nc.vector.tensor_mul(e[:], e[:], keep0[:])
                elif gm == 1:  # GpSimd affine_select path
                    ev = e.rearrange("p (q f) -> p q f", q=4)
                    nc.gpsimd.affine_select(
                        ev, ev, pattern=[[0, 4], [1, TB]],
                        compare_op=mybir.AluOpType.is_ge, fill=0.0,
                        base=-P, channel_multiplier=-1,
                    )
                maybe_fill(pump_every, dummy_ok)
                for q4 in range(4):
                    h = 4 * g + q4
                    cq = (q4 % 2) * 2 + q4 // 2
                    # start=True marks the WHOLE 2KB bank pending-zero, so
                    # only the first matmul touching each av bank may set
                    # it; the sibling column block inherits pending state
                    # and overwrites on its first accumulate.
                    mm(av[q4 // 2][:, (q4 % 2) * TB:(q4 % 2) * TB + TB],
                       vp[i][:, h * VP:h * VP + VP],
                       e[:, cq * TB:(cq + 1) * TB],
                       start=(idx == 0 and q4 % 2 == 0),
                       stop=(idx == ni - 1))
            ux4 = pux.tile([HD + 1, 4 * TB], f32, tag="ux")
            nc.vector.tensor_copy(ux4[:, 0:2 * TB], avA[:])
            nc.vector.tensor_copy(ux4[:, 2 * TB:4 * TB], avB[:])
            ct = pct.tile([1, 4 * TB], f32, tag="ct")
            nc.gpsimd.dma_start(out=ct[:], in_=ux4[HD:HD + 1, :])
            emit_flush(g, jq, ux4, ct)

        def emit_flush(g, jq, ux4, ct):
            # normalize one head group's jq block right after its attV:
            # row 64 of ux4 holds the denominators; reciprocal in place
            # (partition 64), then gpsimd partition_broadcast (AP-relative
            # first partition) fans it out to rows 0..63 -- no DRAM
            # bounce, ~2us chain instead of ~9us.  Per-(g,jq) granularity
            # lets the output projection's k-chain start as soon as each
            # group lands.
            cr = pct.tile([1, 4 * TB], f32, tag="cr")
            nc.vector.reciprocal_approx_fast(cr[:], ct[:])
            bt = pbc.tile([HD, 4 * TB], f32, tag="bt")
            nc.gpsimd.partition_broadcast(bt[:], cr[:])
            col = jq * TB
            for q4 in range(4):
                it = 2 * g + q4 // 2
                cs = q4 * TB
                if q4 % 2 == 0:
                    nc.vector.tensor_mul(
                        outT[it][0:HD, col:col + TB],
                        ux4[0:HD, cs:cs + TB],
                        bt[:, cs:cs + TB],
                    )
                else:
                    nt_ = pbc.tile([HD, TB], mdt, tag="nt")
                    nc.vector.tensor_mul(
                        nt_[:], ux4[0:HD, cs:cs + TB], bt[:, cs:cs + TB])
                    nc.gpsimd.dma_start(
                        out=outT[it][HD:P, col:col + TB], in_=nt_[:])

        # ---- emission schedule ----
        # jq-major rounds: round jq runs head groups g=0..3 then flushes,
        # which fully determines outT columns [512*jq .. ), so the output
        # projection for y m-tiles {2jq, 2jq+1} streams right after.
        # Attention group g needs qT/kT tiles {2g, 2g+1}; round jq needs
        # vp tiles [0, 2jq+2). Projection/out-proj halves ride the filler
        # queue: force() emits hard deps before a unit, pump() fills the
        # PE while ACT works through exp superblocks.
        # All q projections go first: they only need wq+x (the first DMAs),
        # covering the window until wk/wv land.  k/v halves + o halves
        # ride the filler queue: force() emits hard deps before a unit,
        # pump() fills the PE while ACT runs exp.  Each (g, jq) flushes
        # its own outT block immediately, so after g=3 the output
        # projection for y m-tiles {2jq, 2jq+1} is fully determined and
        # its k<6 matmuls overlap the last group's flush chain.
        for m in range(NT):
            for n in (0, 1):
                emit_qk_half("q", m, n)
        for m in range(NT):
            for n in (0, 1):
                enq(("k", m, n), lambda m=m, n=n: emit_qk_half("k", m, n))

        def k_keys(g):
            return [("k", m, n) for m in (2 * g, 2 * g + 1) for n in (0, 1)]

        last = NJ - 1
        for jq in range(NJ):
            # the two v tiles this round unlocks (earlier ones already in)
            for m in (2 * jq, 2 * jq + 1):
                for n in (0, 1):
                    emit_v_half(m, n)
            # pace the filler queue so it lasts the whole round (keeps the
            # PE warm through the ACT-bound late rounds)
            stride = max(1, (4 * (2 * jq + 2)) // max(1, len(fillers)))
            for g in range(4):
                force(k_keys(g))
                if jq == last and g == 3:
                    # partial k<6 chains of y m-tile 6: pumped inside the
                    # last attention unit (its k=6,7 terms need this
                    # unit's flush); keeps the PE busy through its
                    # ACT-paced superblocks.  Only 2 partials may park
                    # (pp_p has 2 bufs) and nothing may pump after them.
                    for n in (0, 1):
                        enq(("op", 6, n),
                            lambda n=n: emit_o_partial(6, n, NT - 2))
                    emit_attn(g, jq, pump_every=2, dummy_ok=True)
                else:
                    emit_attn(g, jq, pump_every=stride, dummy_ok=(jq > 0))
            # o halves for y m-tiles {2jq, 2jq+1} are ready now (all four
            # groups flushed); enqueue a full round late so pumped chains
            # never wait on a flush that just happened
            if jq == 1:
                for m in (0, 1):
                    for n in (0, 1):
                        enq(("o", m, n), lambda m=m, n=n: emit_o_half(m, n))
            elif jq == 2:
                for m in (2, 3, 4, 5):
                    for n in (0, 1):
                        enq(("o", m, n), lambda m=m, n=n: emit_o_half(m, n))
        # bridge the last flush chain with dummy matmuls so the PE array
        # stays busy (and the HAM clock stays at 2.4GHz) until the o6/o7
        # finishers unblock
        for i in range(11):
            emit_dummy()
        drain_all()
        for n in (0, 1):
            emit_o_finish(6, n)
        for n in (0, 1):
            emit_o_half(7, n)
    return nc


def build_program(mm_mode=None):
    """Build + compile the SPMD program once; returns the Bacc object."""
    mode = mm_mode or MM_MODE
    if mode in _COMPILED:
        return _COMPILED[mode]
    import concourse.bacc as bacc
    import concourse.tile as tile
    from concourse import mybir

    mm_dt = {
        "fp32": mybir.dt.float32,
        "fp32r": mybir.dt.float32r,
        "bf16": mybir.dt.bfloat16,
    }[mode]
    nc = bacc.Bacc("TRN2", target_bir_lowering=False, debug=False,
                   num_devices=N_CORES)
    _build(nc, tile, mybir, mm_dt)
    nc.compile()
    _COMPILED[mode] = nc
    return nc


def _np_dt():
    if MM_MODE == "bf16":
        import ml_dtypes
        return ml_dtypes.bfloat16
    return np.float32


def make_in_maps(x, Wk, Wq, Wv, Wo):
    dt = _np_dt()
    wqT = np.ascontiguousarray(np.asarray(Wq, dtype=np.float32).T.astype(dt))
    wkT = np.ascontiguousarray(np.asarray(Wk, dtype=np.float32).T.astype(dt))
    wvT = np.ascontiguousarray(np.asarray(Wv, dtype=np.float32).T.astype(dt))
    woT = np.ascontiguousarray(np.asarray(Wo, dtype=np.float32).T.astype(dt))
    in_maps = []
    for b in range(N_CORES):
        in_maps.append({
            "xT": np.ascontiguousarray(x[b].T.astype(dt)),
            "wqT": wqT, "wkT": wkT, "wvT": wvT, "woT": woT,
        })
    return in_maps


def kernel(x, Wk, Wq, Wv, Wo):
    from concourse.bass_utils import run_bass_kernel_spmd

    x = np.asarray(x, dtype=np.float32)
    nc = build_program()
    in_maps = make_in_maps(x, Wk, Wq, Wv, Wo)
    res = run_bass_kernel_spmd(nc, in_maps, list(range(N_CORES)))
    return np.stack([res.results[c]["y"] for c in range(N_CORES)], axis=0)


# revision 42
# speedup vs baseline: 1.1328x; 1.1328x over previous
"""Causal multi-head self-attention on 8 Trainium2 NeuronCores.

Problem: B=8, T=1024, D=1024, 16 heads (H=64), fp32 in/out, causal softmax,
y = softmax(mask(q k^T)/sqrt(H)) v, then output projection. Weights are
nn.Linear style: q = x @ Wq^T etc.

Sharding: pure data-parallel - one batch element per core, weights
replicated, no collectives.

v2 structure (single fused pipeline, PE kept warm + dense):
  - ~6us of dummy warmup matmuls at t=0 so the PE HAM clock-gate opens
    (2.4GHz) before real work arrives; they overlap the input DMAs.
  - all phases interleaved at (m-tile, 512-half) granularity: attention
    head-group/tq-block units are emitted between projection halves, and
    the output projection for tq block jq streams as soon as all four
    head groups finish that jq block, so y DMAs spread across the kernel
    instead of bunching in a tail.
  - PSUM: 2x[128,1024] score superblocks + 2x[65,512] attV accumulators
    + 2x[128,512] projection slots = 8 banks.
  - attention inner loop consumes each exp'd superblock immediately
    (4 attV matmuls right after the exp), with projection halves pumped
    between superblocks to cover the ACT-engine latency.

Per-core layout (all feature-major, zero on-device transposes); matmul
operands stored in bf16 (fp32 PSUM accumulation), fp32 output:
  host sends xT = x[b].T  [d, t]  and W*T = W*.T  [d_in, d_out]
  qT[do,t] = sum_d WqT[d,do] * xT[d,t]   (lhsT=WqT, rhs=xT)
  kT       likewise
  v[t,do]  = sum_d xT[d,t]  * WvT[d,do]  (lhsT=xT,  rhs=WvT)
  per 4-head group g, per tq block jq of 256, per tk tile i of 128:
    S^T[tk,tq] = sum_hd kT_h[hd,tk] qT_h[hd,tq]  (4 heads in a [128,1024]
                 PSUM superblock; quarters permuted so each 2KB PSUM bank
                 only receives matmuls of ONE PE quadrant position)
    E = exp(S^T/8); diagonal tiles masked (DVE mul with 0/1 mask or
    GpSimd affine_select, alternating)
    attV accumulates vp_h^T E_h over i into [65,512] PSUM pairs
    (vp = [v_h | 1] so row 64 is the softmax denominator)
  outT_h = outX[0:64] * recip(outX[64])  (flash-style denominator,
           broadcast across partitions via a DRAM-bounce DMA)
  y[t,do] = sum_d outT[d,t] * WoT[d,do]
"""

import numpy as np
from collections import deque
from contextlib import ExitStack

N_CORES = 8
T = 1024
D = 1024
NH = 16
HD = 64  # head dim
P = 128
NT = D // P  # 8 tiles of 128 along d or t
TQ = 512     # projection half width
TB = 256     # tq block width in the 4-head-packed attention
NJ = 4       # tq blocks of 256
VP = HD + 1  # v-plus: head cols + ones col
SCALE = 1.0 / 8.0  # 1/sqrt(64)

N_WARMUP = 84   # dummy matmuls at t=0: open the HAM clock gate AND keep
                # the PE continuously busy through the ~19us input-DMA
                # window (a DMA-gated matmul trickle breaks the HAM
                # activity window and leaves the PE at 1.2GHz).  Must be
                # full-width N=512 matmuls: short ones have ~50% PE duty
                # (LDW gaps) and never flip the activity monitor.

# matmul compute dtype: "fp32" (exact, 1/4 rate), "fp32r" (TF32-like, full
# rate), or "bf16" (full rate, half DMA/SBUF traffic)
MM_MODE = "bf16"

_COMPILED = {}


def _build(nc, tile, mybir, mm_dt):
    """Emit the per-core Tile program into nc."""
    f32 = mybir.dt.float32
    Exp = mybir.ActivationFunctionType.Exp

    mdt = mm_dt  # storage dtype for matmul operands
    xT_d = nc.declare_dram_parameter("xT", [D, T], mdt, isOutput=False)
    wqT_d = nc.declare_dram_parameter("wqT", [D, D], mdt, isOutput=False)
    wkT_d = nc.declare_dram_parameter("wkT", [D, D], mdt, isOutput=False)
    wvT_d = nc.declare_dram_parameter("wvT", [D, D], mdt, isOutput=False)
    woT_d = nc.declare_dram_parameter("woT", [D, D], mdt, isOutput=False)
    y_d = nc.declare_dram_parameter("y", [T, D], f32, isOutput=True)

    def mm(out, lhsT, rhs, start, stop):
        nc.tensor.matmul(out, lhsT, rhs, start=start, stop=stop)

    with ExitStack() as ctx:
        tc = ctx.enter_context(tile.TileContext(nc))

        # ---- resident SBUF pools ----
        pqk = ctx.enter_context(tc.tile_pool(name="pqk", bufs=16))
        pv = ctx.enter_context(tc.tile_pool(name="pv", bufs=8))
        pwq = ctx.enter_context(tc.tile_pool(name="pwq", bufs=8))
        pwkv = ctx.enter_context(tc.tile_pool(name="pwkv", bufs=3))
        pxt = ctx.enter_context(tc.tile_pool(name="pxt", bufs=8))
        pout = ctx.enter_context(tc.tile_pool(name="pout", bufs=8))
        pkm = ctx.enter_context(tc.tile_pool(name="pkm", bufs=1))
        pe = ctx.enter_context(tc.tile_pool(name="pe", bufs=4))
        pux = ctx.enter_context(tc.tile_pool(name="pux", bufs=3))
        pct = ctx.enter_context(tc.tile_pool(name="pct", bufs=2))
        pbc = ctx.enter_context(tc.tile_pool(name="pbc", bufs=3))
        py = ctx.enter_context(tc.tile_pool(name="py", bufs=2))

        qT = [pqk.tile([P, T], mdt, tag="qk", name=f"qT{i}") for i in range(NT)]
        kT = [pqk.tile([P, T], mdt, tag="qk", name=f"kT{i}") for i in range(NT)]
        vp = [pv.tile([P, NH * VP], mdt, tag="vp", name=f"vp{i}")
              for i in range(NT)]
        wq_sb = [pwq.tile([P, D], mdt, tag="wq", name=f"wq{i}")
                 for i in range(NT)]
        wk_all = pwkv.tile([P, NT * D], mdt, tag="wkv", name="wk_all")
        wv_all = pwkv.tile([P, NT * D], mdt, tag="wkv", name="wv_all")
        wo_all = pwkv.tile([P, NT * D], mdt, tag="wkv", name="wo_all")
        xT = [pxt.tile([P, T], mdt, tag="xt", name=f"xTs{i}")
              for i in range(NT)]
        outT = [pout.tile([P, T], mdt, tag="ot", name=f"outT{i}")
                for i in range(NT)]

        # ---- warmup: open the HAM clock gate while input DMAs run ----
        # (emitted first so the PE is at 2.4GHz when real matmuls start;
        # reads a memset tile, writes a scratch PSUM bank that the stack
        # allocator reuses afterwards)
        with (
            tc.tile_pool(name="pwarm", bufs=1) as pwarm,
            tc.tile_pool(name="pwarm_ps", bufs=2, space="PSUM") as pwarm_ps,
        ):
            wt = pwarm.tile([P, P + TQ], mdt, name="warm_in")
            nc.gpsimd.memset(wt[:], 0.0)
            wp = [pwarm_ps.tile([P, TQ], f32, tag="wps", name=f"wps{i}")
                  for i in range(2)]
            for i in range(N_WARMUP):
                mm(wp[i % 2][:], wt[:, 0:P], wt[:, P:P + TQ],
                   start=True, stop=True)

        # ---- input DMAs (dependency-first order) ----
        nc.sync.dma_start(out=wq_sb[0][:, 0:P], in_=wqT_d[0:P, 0:P])
        nc.sync.dma_start(out=xT[0][:, 0:TQ], in_=xT_d[0:P, 0:TQ])
        nc.sync.dma_start(out=wq_sb[0][:, P:D], in_=wqT_d[0:P, P:D])
        nc.sync.dma_start(out=xT[0][:, TQ:T], in_=xT_d[0:P, TQ:T])
        for k in range(1, NT):
            nc.sync.dma_start(out=wq_sb[k][:], in_=wqT_d[k * P:(k + 1) * P, :])
            nc.sync.dma_start(out=xT[k][:], in_=xT_d[k * P:(k + 1) * P, :])
        nc.sync.dma_start(
            out=wk_all.rearrange("p (k d) -> p k d", d=D),
            in_=wkT_d.rearrange("(k p) d -> p k d", p=P),
        )
        nc.sync.dma_start(
            out=wv_all.rearrange("p (k d) -> p k d", d=D),
            in_=wvT_d.rearrange("(k p) d -> p k d", p=P),
        )
        nc.sync.dma_start(
            out=wo_all.rearrange("p (k d) -> p k d", d=D),
            in_=woT_d.rearrange("(k p) d -> p k d", p=P),
        )

        # ---- 0/1 keep-mask for gm=0 diagonal tiles ----
        keep0 = pkm.tile([P, 4 * TB], mdt, name="keep0")
        nc.gpsimd.memset(keep0[:], 1.0)
        kv = keep0.rearrange("p (q f) -> p q f", q=4)
        nc.gpsimd.affine_select(
            kv, kv, pattern=[[0, 4], [1, TB]],
            compare_op=mybir.AluOpType.is_ge, fill=0.0,
            base=0, channel_multiplier=-1,
        )
        # ones columns of v-plus (denominator rows for the attV matmul)
        for m in range(NT):
            ones_cols = vp[m].rearrange("p (h c) -> p h c", c=VP)[:, :, VP - 1]
            nc.gpsimd.memset(ones_cols, 1.0)

        # ---- PSUM pools (8 banks total) ----
        pp_s = ctx.enter_context(
            tc.tile_pool(name="pp_s", bufs=2, space="PSUM"))
        pp_av = ctx.enter_context(
            tc.tile_pool(name="pp_av", bufs=2, space="PSUM"))
        pp_p = ctx.enter_context(
            tc.tile_pool(name="pp_p", bufs=2, space="PSUM"))

        # ---- copy-engine rotation for PSUM->SBUF drains ----
        cp_state = {"i": 0}

        def drain_copy(dst, src):
            # gpsimd cannot read PSUM: rotate vector/scalar 3:1
            r = cp_state["i"] % 4
            cp_state["i"] += 1
            if r == 2:
                nc.scalar.copy(dst, src)
            else:
                nc.vector.tensor_copy(dst, src)

        # ---- work-unit emitters ----
        def emit_qk_half(which, m, n):
            ps = pp_p.tile([P, TQ], f32, tag="pp")
            for k in range(NT):
                if which == "q":
                    lhsT = wq_sb[k][:, m * P:(m + 1) * P]
                else:
                    lhsT = wk_all[:, k * D + m * P:k * D + (m + 1) * P]
                mm(ps[:], lhsT, xT[k][:, n * TQ:(n + 1) * TQ],
                   start=(k == 0), stop=(k == NT - 1))
            dst = (qT if which == "q" else kT)[m][:, n * TQ:(n + 1) * TQ]
            drain_copy(dst, ps[:])

        def emit_v_half(m, n):
            # v natural layout: out rows = t tile m, cols = d half n
            ps = pp_p.tile([P, TQ], f32, tag="pp")
            for k in range(NT):
                mm(ps[:], xT[k][:, m * P:(m + 1) * P],
                   wv_all[:, k * D + n * TQ:k * D + (n + 1) * TQ],
                   start=(k == 0), stop=(k == NT - 1))
            hpn = TQ // HD  # heads per 512-half
            vdst = vp[m].rearrange("p (h c) -> p h c", c=VP)[
                :, n * hpn:(n + 1) * hpn, 0:HD]
            vsrc = ps.rearrange("p (h c) -> p h c", c=HD)
            nc.vector.tensor_copy(vdst, vsrc)

        o_open = {}

        def emit_o_mms(ps, m, n, kfrom, kend):
            for k in range(kfrom, kend):
                mm(ps[:], outT[k][:, m * P:(m + 1) * P],
                   wo_all[:, k * D + n * TQ:k * D + (n + 1) * TQ],
                   start=(k == 0), stop=(k == NT - 1))

        def emit_o_partial(m, n, kend):
            # open the y (m,n) psum and accumulate k < kend; the chain
            # parks in its pp_p buf so the k terms gated on the last head
            # group can finish after its flush without idling the PE
            ps = pp_p.tile([P, TQ], f32, tag="pp")
            emit_o_mms(ps, m, n, 0, kend)
            o_open[(m, n)] = (ps, kend)

        def emit_o_drain(ps, m, n):
            ysb = py.tile([P, TQ], f32, tag="y")
            nc.vector.tensor_copy(ysb[:], ps[:])
            nc.sync.dma_start(
                out=y_d[m * P:(m + 1) * P, n * TQ:(n + 1) * TQ], in_=ysb[:])

        def emit_o_finish(m, n):
            ps, kend = o_open.pop((m, n))
            emit_o_mms(ps, m, n, kend, NT)
            emit_o_drain(ps, m, n)

        def emit_o_half(m, n):
            ps = pp_p.tile([P, TQ], f32, tag="pp")
            emit_o_mms(ps, m, n, 0, NT)
            emit_o_drain(ps, m, n)

        # ---- filler queue: half-units pumped between attention blocks ----
        # queue holds (key, thunk); force() emits specific keys a unit
        # depends on, pump() emits from the front to fill PE gaps while
        # the ACT engine works through the exp superblocks.
        fillers = deque()
        emitted = set()

        def enq(key, thunk):
            fillers.append((key, thunk))

        fill_ctr = {"n": 0}

        def emit_dummy():
            # 2 full-width matmuls into a scores-pool bank: pure PE-array
            # activity to hold the HAM clock gate open through ACT-bound
            # stretches; the result is never read
            dps = pp_s.tile([P, 4 * TB], f32, tag="ps", name="dummy_ps")
            for _ in range(2):
                mm(dps[:, 0:TQ], keep0[:, 0:P], keep0[:, P:P + TQ],
                   start=True, stop=True)

        def maybe_fill(stride, dummy_ok):
            # once per superblock: every stride-th slot emits a real
            # filler half; other slots emit standalone weight loads when
            # allowed -- PE-array activity for the HAM clock gate that
            # needs no PSUM bank (an MM dummy would steal a pp_s slot and
            # collapse the scores double-buffering)
            fill_ctr["n"] += 1
            if fillers and fill_ctr["n"] % stride == 0:
                key, thunk = fillers.popleft()
                emitted.add(key)
                thunk()
            elif dummy_ok:
                for _ in range(6):
                    nc.tensor.ldweights(keep0[:, 0:P])

        def pump(k=1):
            for _ in range(k):
                if not fillers:
                    return
                key, thunk = fillers.popleft()
                emitted.add(key)
                thunk()

        def force(keys):
            keys = [k for k in keys if k not in emitted]
            if not keys:
                return
            want = set(keys)
            rest = deque()
            while fillers and want:
                key, thunk = fillers.popleft()
                if key in want:
                    emitted.add(key)
                    thunk()
                    want.discard(key)
                else:
                    rest.append((key, thunk))
            fillers.extendleft(reversed(rest))
            assert not want, f"missing filler deps: {want}"

        def drain_all():
            pump(len(fillers))



        def emit_attn(g, jq, pump_every=1, dummy_ok=False):
            ni = 2 * jq + 2
            avA = pp_av.tile([HD + 1, 2 * TB], f32, tag="av")
            avB = pp_av.tile([HD + 1, 2 * TB], f32, tag="av")
            av = (avA, avB)
            order = list(range(ni - 1, -1, -1))  # diagonal tiles first
            for idx, i in enumerate(order):
                ps = pp_s.tile([P, 4 * TB], f32, tag="ps")
                gm = i - 2 * jq
                for q4 in range(4):
                    it = 2 * g + q4 // 2
                    po = (q4 % 2) * HD
                    # same-quadrant matmuls share a PSUM bank: bank0 gets
                    # the two po=0 heads, bank1 the po=64 heads (mixed
                    # quadrant positions in one bank fault the hardware)
                    cs = ((q4 % 2) * 2 + q4 // 2) * TB
                    mm(ps[:, cs:cs + TB],
                       kT[it][po:po + HD, i * P:(i + 1) * P],
                       qT[it][po:po + HD, jq * TB:(jq + 1) * TB],
                       start=True, stop=True)
                e = pe.tile([P, 4 * TB], mdt, tag="e")
                nc.scalar.activation(e[:], ps[:], Exp, scale=SCALE)
                if gm == 0:  # DVE multiply path
                    nc.vector.tensor_mul(e[:], e[:], keep0[:])
                elif gm == 1:  # GpSimd affine_select path
                    ev = e.rearrange("p (q f) -> p q f", q=4)
                    nc.gpsimd.affine_select(
                        ev, ev, pattern=[[0, 4], [1, TB]],
                        compare_op=mybir.AluOpType.is_ge, fill=0.0,
                        base=-P, channel_multiplier=-1,
                    )
                maybe_fill(pump_every, dummy_ok)
                for q4 in range(4):
                    h = 4 * g + q4
                    cq = (q4 % 2) * 2 + q4 // 2
                    # start=True marks the WHOLE 2KB bank pending-zero, so
                    # only the first matmul touching each av bank may set
                    # it; the sibling column block inherits pending state
                    # and overwrites on its first accumulate.
                    mm(av[q4 // 2][:, (q4 % 2) * TB:(q4 % 2) * TB + TB],
                       vp[i][:, h * VP:h * VP + VP],
                       e[:, cq * TB:(cq + 1) * TB],
                       start=(idx == 0 and q4 % 2 == 0),
                       stop=(idx == ni - 1))
            ux4 = pux.tile([HD + 1, 4 * TB], f32, tag="ux")
            nc.vector.tensor_copy(ux4[:, 0:2 * TB], avA[:])
            nc.vector.tensor_copy(ux4[:, 2 * TB:4 * TB], avB[:])
            ct = pct.tile([1, 4 * TB], f32, tag="ct")
            nc.gpsimd.dma_start(out=ct[:], in_=ux4[HD:HD + 1, :])
            emit_flush(g, jq, ux4, ct)

        def emit_flush(g, jq, ux4, ct):
            # normalize one head group's jq block right after its attV:
            # row 64 of ux4 holds the denominators; reciprocal in place
            # (partition 64), then gpsimd partition_broadcast (AP-relative
            # first partition) fans it out to rows 0..63 -- no DRAM
            # bounce, ~2us chain instead of ~9us.  Per-(g,jq) granularity
            # lets the output projection's k-chain start as soon as each
            # group lands.
            cr = pct.tile([1, 4 * TB], f32, tag="cr")
            nc.vector.reciprocal_approx_fast(cr[:], ct[:])
            bt = pbc.tile([HD, 4 * TB], f32, tag="bt")
            nc.gpsimd.partition_broadcast(bt[:], cr[:])
            col = jq * TB
            for q4 in range(4):
                it = 2 * g + q4 // 2
                cs = q4 * TB
                if q4 % 2 == 0:
                    nc.vector.tensor_mul(
                        outT[it][0:HD, col:col + TB],
                        ux4[0:HD, cs:cs + TB],
                        bt[:, cs:cs + TB],
                    )
                else:
                    nt_ = pbc.tile([HD, TB], mdt, tag="nt")
                    nc.vector.tensor_mul(
                        nt_[:], ux4[0:HD, cs:cs + TB], bt[:, cs:cs + TB])
                    nc.gpsimd.dma_start(
                        out=outT[it][HD:P, col:col + TB], in_=nt_[:])

        # ---- emission schedule ----
        # jq-major rounds: round jq runs head groups g=0..3 then flushes,
        # which fully determines outT columns [512*jq .. ), so the output
        # projection for y m-tiles {2jq, 2jq+1} streams right after.
        # Attention group g needs qT/kT tiles {2g, 2g+1}; round jq needs
        # vp tiles [0, 2jq+2). Projection/out-proj halves ride the filler
        # queue: force() emits hard deps before a unit, pump() fills the
        # PE while ACT works through exp superblocks.
        # All q projections go first: they only need wq+x (the first DMAs),
        # covering the window until wk/wv land.  k/v halves + o halves
        # ride the filler queue: force() emits hard deps before a unit,
        # pump() fills the PE while ACT runs exp.  Each (g, jq) flushes
        # its own outT block immediately, so after g=3 the output
        # projection for y m-tiles {2jq, 2jq+1} is fully determined and
        # its k<6 matmuls overlap the last group's flush chain.
        for m in range(NT):
            for n in (0, 1):
                emit_qk_half("q", m, n)
        for m in range(NT):
            for n in (0, 1):
                enq(("k", m, n), lambda m=m, n=n: emit_qk_half("k", m, n))

        def k_keys(g):
            return [("k", m, n) for m in (2 * g, 2 * g + 1) for n in (0, 1)]

        last = NJ - 1
        for jq in range(NJ):
            # the two v tiles this round unlocks (earlier ones already in)
            for m in (2 * jq, 2 * jq + 1):
                for n in (0, 1):
                    emit_v_half(m, n)
            # pace the filler queue so it lasts the whole round (keeps the
            # PE warm through the ACT-bound late rounds)
            stride = max(1, (4 * (2 * jq + 2)) // max(1, len(fillers)))
            for g in range(4):
                force(k_keys(g))
                if jq == last and g == 3:
                    # partial k<6 chains of y m-tile 6: pumped inside the
                    # last attention unit (its k=6,7 terms need this
                    # unit's flush); keeps the PE busy through its
                    # ACT-paced superblocks.  Only 2 partials may park
                    # (pp_p has 2 bufs) and nothing may pump after them.
                    for n in (0, 1):
                        enq(("op", 6, n),
                            lambda n=n: emit_o_partial(6, n, NT - 2))
                    emit_attn(g, jq, pump_every=2, dummy_ok=True)
                else:
                    emit_attn(g, jq, pump_every=stride, dummy_ok=(jq > 0))
            # o halves for y m-tiles {2jq, 2jq+1} are ready now (all four
            # groups flushed); enqueue a full round late so pumped chains
            # never wait on a flush that just happened
            if jq == 1:
                for m in (0, 1):
                    for n in (0, 1):
                        enq(("o", m, n), lambda m=m, n=n: emit_o_half(m, n))
            elif jq == 2:
                for m in (2, 3, 4, 5):
                    for n in (0, 1):
                        enq(("o", m, n), lambda m=m, n=n: emit_o_half(m, n))
        # bridge the last flush chain with dummy matmuls so the PE array
        # stays busy (and the HAM clock stays at 2.4GHz) until the o6/o7
        # finishers unblock
        for i in range(11):
            emit_dummy()
        drain_all()
        for n in (0, 1):
            emit_o_finish(6, n)
        for n in (0, 1):
            emit_o_half(7, n)
    return nc


def build_program(mm_mode=None):
    """Build + compile the SPMD program once; returns the Bacc object."""
    mode = mm_mode or MM_MODE
    if mode in _COMPILED:
        return _COMPILED[mode]
    import concourse.bacc as bacc
    import concourse.tile as tile
    from concourse import mybir

    mm_dt = {
        "fp32": mybir.dt.float32,
        "fp32r": mybir.dt.float32r,
        "bf16": mybir.dt.bfloat16,
    }[mode]
    nc = bacc.Bacc("TRN2", target_bir_lowering=False, debug=False,
                   num_devices=N_CORES)
    _build(nc, tile, mybir, mm_dt)
    nc.compile()
    _COMPILED[mode] = nc
    return nc


def _np_dt():
    if MM_MODE == "bf16":
        import ml_dtypes
        return ml_dtypes.bfloat16
    return np.float32


def make_in_maps(x, Wk, Wq, Wv, Wo):
    dt = _np_dt()
    wqT = np.ascontiguousarray(np.asarray(Wq, dtype=np.float32).T.astype(dt))
    wkT = np.ascontiguousarray(np.asarray(Wk, dtype=np.float32).T.astype(dt))
    wvT = np.ascontiguousarray(np.asarray(Wv, dtype=np.float32).T.astype(dt))
    woT = np.ascontiguousarray(np.asarray(Wo, dtype=np.float32).T.astype(dt))
    in_maps = []
    for b in range(N_CORES):
        in_maps.append({
            "xT": np.ascontiguousarray(x[b].T.astype(dt)),
            "wqT": wqT, "wkT": wkT, "wvT": wvT, "woT": woT,
        })
    return in_maps


def kernel(x, Wk, Wq, Wv, Wo):
    from concourse.bass_utils import run_bass_kernel_spmd

    x = np.asarray(x, dtype=np.float32)
    nc = build_program()
    in_maps = make_in_maps(x, Wk, Wq, Wv, Wo)
    res = run_bass_kernel_spmd(nc, in_maps, list(range(N_CORES)))
    return np.stack([res.results[c]["y"] for c in range(N_CORES)], axis=0)
